# revision 17
# baseline (speedup 1.0000x reference)
"""Causal self-attention Bass kernel for TRN2, 8-core batch x head-group sharding.

Per-core computation (core c: batch b=c//4, head-group g=c%4, 4 heads):
  inputs (pre-transposed, bf16, prepared on host):
    xT   [1024, 2048]  = x[b].T
    wqT  [1024, 256]   = w_q[256g:256g+256, :].T
    wkT  [1024, 256]
    wvT  [1024, 256]
    woT  [256, 1024]   = w_o[:, 256g:256g+256].T
    mask [128, 128]    upper-tri (incl diag) ones, bf16
  output:
    o    [2048, 1024]  bf16 partial (host sums 4 partials per batch in f32)

Device schedule (engine-balanced, PE kept continuously busy):
  load:   q/k pair-0 projections accumulate ct-OUTER into 8 psum regions
          spanning all 8 banks, so each arriving xT chunk feeds 8 matmuls
          immediately (PE busy during the xT DMA stream).
  v+A0:   v-proj st tiles 4sqb..4sqb+3 emitted just before attention(p0,sqb);
          q/k pair-1 projections fill PE while ACT runs pair-0 exps.
  A1+O:   attention pair 1 with the output projection trailing one sqb.
  postproc per (pair, sqb): rowsum row 64 of av psum -> DVE reciprocal
          (bf16 out) -> DRAM-roundtrip broadcast [64, 1024] bf16 -> Pool TT
          multiply with the bf16-evicted av -> attnT. Mask TT also on Pool.
  scores: S^T[sk, sq] = matmul(lhsT=kT, rhs=qT) two heads row-packed; one
          ACT exp per (t, both heads); av accumulated with [v_h | 1] lhsT so
          row 64 is the softmax rowsum.
"""

from contextlib import ExitStack

import numpy as np
import ml_dtypes

import concourse.bass as bass
import concourse.mybir as mybir
import concourse.tile as tile

F32 = mybir.dt.float32
BF16 = mybir.dt.bfloat16
AF = mybir.ActivationFunctionType
ALU = mybir.AluOpType

D_MODEL = 1024
N_HEADS = 16
HEAD_DIM = 64
BATCH = 2
SEQ = 2048
N_CORES = 8
HG = 4               # heads per core
DG = HG * HEAD_DIM   # 256 projection dims per core
P = 128
SQB = 512            # sq chunk width
N_SQB = SEQ // SQB   # 4
N_KT = SEQ // P      # 16 sk tiles
N_CT = D_MODEL // P  # 8 contraction tiles
SCALE = 1.0 / np.sqrt(HEAD_DIM)

MAX_WAITS = 1  # this walrus supports a single sync wait per instruction


def split_excess_waits(nc):
    """This container's walrus supports 1 sync wait per instruction. Move
    extras onto NoOps inserted immediately before, on the same engine."""
    n_split = 0
    for b in nc.m.functions[0].blocks:
        insts = b.instructions
        i = 0
        while i < len(insts):
            inst = insts[i]
            si = inst.sync_info
            if si is None or si.on_wait is None or len(si.on_wait) <= MAX_WAITS:
                i += 1
                continue
            waits = list(si.on_wait)
            si.on_wait = waits[:MAX_WAITS]
            extra = waits[MAX_WAITS:]
            pos = i
            for j in range(0, len(extra), MAX_WAITS):
                no = mybir.InstNoOp(
                    name=f"{inst.name}_wsplit{n_split}",
                    engine=inst.engine,
                    sync_info=mybir.SyncInfo(
                        on_wait=extra[j : j + MAX_WAITS], on_update=[]
                    ),
                )
                insts.insert(pos, no)
                pos += 1
                n_split += 1
                i += 1
            i += 1
    return n_split


def build_kernel(split_waits=True, repeat=1):
    nc = bass.Bass("TRN2")
    xT = nc.dram_tensor("xT", [D_MODEL, SEQ], BF16, kind="ExternalInput")
    wqT = nc.dram_tensor("wqT", [D_MODEL, DG], BF16, kind="ExternalInput")
    wkT = nc.dram_tensor("wkT", [D_MODEL, DG], BF16, kind="ExternalInput")
    wvT = nc.dram_tensor("wvT", [D_MODEL, DG], BF16, kind="ExternalInput")
    woT = nc.dram_tensor("woT", [DG, D_MODEL], BF16, kind="ExternalInput")
    mask = nc.dram_tensor("mask", [P, P], BF16, kind="ExternalInput")
    o = nc.dram_tensor("o", [SEQ, D_MODEL], BF16, kind="ExternalOutput")

    with ExitStack() as ctx:
        tc = ctx.enter_context(tile.TileContext(nc))
        build_body(ctx, tc, xT, wqT, wkT, wvT, woT, mask, o, repeat=repeat)

    if split_waits:
        split_excess_waits(nc)
    return nc


def build_body(ctx, tc, xT, wqT, wkT, wvT, woT, mask, o, repeat=1):
    nc = tc.nc

    consts = ctx.enter_context(tc.tile_pool(name="consts", bufs=1))
    persist = ctx.enter_context(tc.tile_pool(name="persist", bufs=1))
    pt_pool = ctx.enter_context(tc.tile_pool(name="pt", bufs=8))
    av_pool = ctx.enter_context(tc.tile_pool(name="avp", bufs=3))
    rs_pool = ctx.enter_context(tc.tile_pool(name="rs", bufs=4))
    rsb_pool = ctx.enter_context(tc.tile_pool(name="rsb", bufs=3))
    dram = ctx.enter_context(tc.tile_pool(name="dram", bufs=4, space="DRAM"))
    # PSUM budget (8 banks of [128, 512]f32):
    #   s (scores, merged heads) [128,1024] x2 bufs = 4 banks
    #   av (merged heads)        [128,1024] x1 buf  = 2 banks
    #   pp (proj A + oproj C)    [128,512]  x2 bufs = 2 banks
    # The load phase borrows ALL of them for 8 ct-outer q/k accumulators.
    psum_s = ctx.enter_context(tc.tile_pool(name="psum_s", bufs=2, space="PSUM"))
    psum_av = ctx.enter_context(tc.tile_pool(name="psum_av", bufs=1, space="PSUM"))
    psum_p = ctx.enter_context(tc.tile_pool(name="psum_p", bufs=2, space="PSUM"))

    # ---- persistent SBUF tensors
    xT_sb = []
    for ct in range(N_CT):
        xt_t = persist.tile([P, SEQ], BF16, tag=f"xT{ct}", name=f"xT{ct}")
        xT_sb.append(xt_t)
    wqT_sb = persist.tile([P, N_CT, DG], BF16)
    wkT_sb = persist.tile([P, N_CT, DG], BF16)
    wvT_sb = persist.tile([P, N_CT, DG], BF16)
    woT_sb = persist.tile([P, 2, D_MODEL], BF16)
    qT_sb = persist.tile([P, 2, SEQ], BF16)
    kT_sb = persist.tile([P, 2, SEQ], BF16)
    v_sb = persist.tile([P, N_KT, HG, HEAD_DIM + 1], BF16)
    attnT_sb = persist.tile([P, 2, SEQ], BF16)
    o_sb = persist.tile([P, N_KT, D_MODEL], BF16)
    trimask = consts.tile([P, P], BF16)

    # ---- input DMAs (ordered so the first load-phase matmuls start early;
    # w_q/w_k split into pair halves so pair 0 lands first)
    wq3 = wqT.rearrange("(ct p) d -> p ct d", p=P)
    wk3 = wkT.rearrange("(ct p) d -> p ct d", p=P)
    xT3 = xT.rearrange("(ct p) s -> ct p s", p=P)
    nc.sync.dma_start(wqT_sb[:, :, 0:P], wq3[:, :, 0:P])
    nc.sync.dma_start(xT_sb[0][:], xT3[0])
    nc.sync.dma_start(wkT_sb[:, :, 0:P], wk3[:, :, 0:P])
    nc.sync.dma_start(trimask[:], mask[:])
    for ct in range(1, N_CT):
        nc.sync.dma_start(xT_sb[ct][:], xT3[ct])
    nc.sync.dma_start(wvT_sb[:], wvT.rearrange("(ct p) d -> p ct d", p=P))
    nc.sync.dma_start(woT_sb[:], woT.rearrange("(ct p) d -> p ct d", p=P))
    nc.sync.dma_start(wqT_sb[:, :, P:DG], wq3[:, :, P:DG])
    nc.sync.dma_start(wkT_sb[:, :, P:DG], wk3[:, :, P:DG])

    # ones column in v (lhsT = [v_h | 1] makes av row 64 the softmax rowsum)
    nc.vector.memset(v_sb[:, :, :, HEAD_DIM], 1.0)
    # ones row for the K=1 broadcast matmul in the final postproc
    ones_bf = consts.tile([1, 64], BF16, tag="ones")
    nc.vector.memset(ones_bf[:], 1.0)

    def load_phase():
        """q/k pair-0 projections, ct-outer into 8 psum accumulator regions
        spanning all 8 banks. Each arriving xT chunk immediately feeds 8
        matmuls, overlapping PE with the input DMA stream."""
        sA = psum_s.tile([P, 2 * SQB], F32, tag="s", name="ld_qA")
        sB = psum_s.tile([P, 2 * SQB], F32, tag="s", name="ld_qB")
        kAV = psum_av.tile([P, 2 * SQB], F32, tag="av", name="ld_kAV")
        kp = [
            psum_p.tile([P, SQB], F32, tag="pp", name=f"ld_kp{i}") for i in range(2)
        ]
        q_dst = [sA[:, 0:SQB], sA[:, SQB:], sB[:, 0:SQB], sB[:, SQB:]]
        k_dst = [kAV[:, 0:SQB], kAV[:, SQB:], kp[0][:], kp[1][:]]
        for ct in range(N_CT):
            st = ct == 0
            sp = ct == N_CT - 1
            for sqb in range(N_SQB):
                nc.tensor.matmul(
                    q_dst[sqb],
                    lhsT=wqT_sb[:, ct, 0:P],
                    rhs=xT_sb[ct][:, sqb * SQB : (sqb + 1) * SQB],
                    start=st,
                    stop=sp,
                    skip_group_check=True,
                )
            for sqb in range(N_SQB):
                nc.tensor.matmul(
                    k_dst[sqb],
                    lhsT=wkT_sb[:, ct, 0:P],
                    rhs=xT_sb[ct][:, sqb * SQB : (sqb + 1) * SQB],
                    start=st,
                    stop=sp,
                    skip_group_check=True,
                )
        # evictions: pp tiles first (proj_v needs those psum bufs soonest),
        # interleaved across ACT and DVE
        nc.vector.tensor_copy(kT_sb[:, 0, 2 * SQB : 3 * SQB], kp[0][:])
        nc.scalar.copy(kT_sb[:, 0, 3 * SQB : 4 * SQB], kp[1][:])
        nc.scalar.copy(qT_sb[:, 0, 0 : 2 * SQB], sA[:])
        nc.vector.tensor_copy(qT_sb[:, 0, 2 * SQB : 4 * SQB], sB[:])
        nc.scalar.copy(kT_sb[:, 0, 0 : 2 * SQB], kAV[:])

    class Filler:
        """Ordered chain of (tag, generator) units drawn one instruction at a
        time between attention tiles to keep PE fed while ACT runs exps."""

        def __init__(self, items):
            self.items = list(items)
            self.i = 0

        def draw(self, max_tag, n=1):
            emitted = 0
            while emitted < n and self.i < len(self.items):
                tag, gen = self.items[self.i]
                if tag > max_tag:
                    return emitted
                try:
                    next(gen)
                    emitted += 1
                except StopIteration:
                    self.i += 1
            return emitted

        def drain_tag(self, max_tag):
            while self.i < len(self.items) and self.items[self.i][0] <= max_tag:
                for _ in self.items[self.i][1]:
                    pass
                self.i += 1

    def gen_proj_qk(w_sb, out_sb, pair, sqb):
        ps = psum_p.tile([P, SQB], F32, tag="pp", name="ps_qk")
        for ct in range(N_CT):
            nc.tensor.matmul(
                ps[:],
                lhsT=w_sb[:, ct, pair * P : (pair + 1) * P],
                rhs=xT_sb[ct][:, sqb * SQB : (sqb + 1) * SQB],
                start=(ct == 0),
                stop=(ct == N_CT - 1),
            )
            yield
        nc.vector.tensor_copy(out_sb[:, pair, sqb * SQB : (sqb + 1) * SQB], ps[:])
        yield

    def gen_proj_v(st):
        ps = psum_p.tile([P, SQB], F32, tag="pp", name="ps_v")
        for ct in range(N_CT):
            nc.tensor.matmul(
                ps[:, 0:DG],
                lhsT=xT_sb[ct][:, st * P : (st + 1) * P],
                rhs=wvT_sb[:, ct, :],
                start=(ct == 0),
                stop=(ct == N_CT - 1),
            )
            yield
        nc.vector.tensor_copy(
            v_sb[:, st, :, 0:HEAD_DIM],
            ps[:, 0:DG].rearrange("p (h d) -> p h d", h=HG),
        )
        yield

    def gen_v_group(sts):
        for st in sts:
            yield from gen_proj_v(st)

    def attention_sqb(pair, sqb, last=False, filler=None, tag_fn=None):
        av = psum_av.tile([P, 2 * SQB], F32, tag="av", name="av")
        for t in range(4 * sqb + 4):
            r = t - 4 * sqb  # >= 0 on the diagonal tile
            off = max(0, r * P)
            w = SQB - off
            sq0 = sqb * SQB + off
            ss = psum_s.tile([P, 2 * SQB], F32, tag="s", name="ss")
            for i in range(2):  # head-in-pair, PE rows i*64..i*64+63
                nc.tensor.matmul(
                    ss[:, i * SQB + off : (i + 1) * SQB],
                    lhsT=kT_sb[i * 64 : (i + 1) * 64, pair, t * P : (t + 1) * P],
                    rhs=qT_sb[i * 64 : (i + 1) * 64, pair, sq0 : sq0 + w],
                    start=True,
                    stop=True,
                )
            pt = pt_pool.tile([P, 2, SQB], BF16, tag="pt", name="pt")
            ss2 = ss[:].rearrange("p (i n) -> p i n", i=2)
            nc.scalar.activation(
                pt[:, :, off:SQB], ss2[:, :, off:SQB], AF.Exp, scale=SCALE
            )
            if r >= 0:
                nc.gpsimd.tensor_tensor(
                    pt[:, :, off : off + P],
                    pt[:, :, off : off + P],
                    trimask[:, None, :].to_broadcast([P, 2, P]),
                    ALU.mult,
                )
            for i in range(2):
                h = 2 * pair + i
                nc.tensor.matmul(
                    av[0 : HEAD_DIM + 1, i * SQB + off : (i + 1) * SQB],
                    lhsT=v_sb[:, t, h, :],
                    rhs=pt[:, i, off:SQB],
                    start=(t == 0),
                    stop=(t == 4 * sqb + 3),
                    skip_group_check=True,
                )
            if filler is not None:
                filler.draw(tag_fn(t) if tag_fn else 99, 1)
        # postproc: recip straight from the PSUM rowsum row (DVE) in parallel
        # with the bf16 av eviction (ACT) — av bank frees and the broadcast
        # roundtrip starts ~1.1us earlier than a serial chain; normalize TT
        # on the Pool engine (all-bf16 operands).
        rs_bf = rs_pool.tile([1, 2 * SQB], BF16, tag="rsrow", name="rs_bf")
        with nc.allow_low_precision(reason="rowsum reciprocal in bf16"):
            nc.vector.reciprocal(
                rs_bf[0:1, :], av[HEAD_DIM : HEAD_DIM + 1, :]
            )
        av_sb = av_pool.tile([HEAD_DIM, 2 * SQB], BF16, tag="avsb", name="av_sb")
        nc.scalar.copy(av_sb[:], av[0:HEAD_DIM, :])
        if last:
            # final block: the DMA roundtrip would sit exposed on the kernel
            # tail — broadcast via a K=1 PE matmul instead (PE is idle here)
            # and multiply on DVE with the broadcast read from PSUM.
            rs_ps = psum_s.tile([P, 2 * SQB], F32, tag="s", name="rs_ps")
            for i in range(2):  # one matmul per head: psum-bank-sized outputs
                nc.tensor.matmul(
                    rs_ps[0:64, i * SQB : (i + 1) * SQB],
                    lhsT=ones_bf[:],
                    rhs=rs_bf[0:1, i * SQB : (i + 1) * SQB],
                    start=True,
                    stop=True,
                )
            for i in range(2):
                nc.vector.tensor_tensor(
                    attnT_sb[
                        i * 64 : (i + 1) * 64, pair, sqb * SQB : (sqb + 1) * SQB
                    ],
                    av_sb[0:HEAD_DIM, i * SQB : (i + 1) * SQB],
                    rs_ps[0:64, i * SQB : (i + 1) * SQB],
                    ALU.mult,
                )
            return
        rsdi = dram.tile([1, 2 * SQB], BF16, tag="rsdi", name="rsdi")
        nc.sync.dma_start(rsdi[:], rs_bf[0:1, :])
        rs_b = rsb_pool.tile([64, 2 * SQB], BF16, tag="rsb", name="rs_b")
        nc.sync.dma_start(rs_b[:], rsdi[0, None, :].to_broadcast([64, 2 * SQB]))
        for i in range(2):
            nc.gpsimd.tensor_tensor(
                attnT_sb[i * 64 : (i + 1) * 64, pair, sqb * SQB : (sqb + 1) * SQB],
                av_sb[0:HEAD_DIM, i * SQB : (i + 1) * SQB],
                rs_b[:, i * SQB : (i + 1) * SQB],
                ALU.mult,
            )

    o3 = o.rearrange("(q p) d -> p q d", p=P)

    def gen_oproj_qt(qt):
        for dc in range(2):
            ps = psum_p.tile([P, SQB], F32, tag="pp", name="ps_o")
            for pair in range(2):
                nc.tensor.matmul(
                    ps[:],
                    lhsT=attnT_sb[:, pair, qt * P : (qt + 1) * P],
                    rhs=woT_sb[:, pair, dc * SQB : (dc + 1) * SQB],
                    start=(pair == 0),
                    stop=(pair == 1),
                )
                yield
            nc.vector.tensor_copy(o_sb[:, qt, dc * SQB : (dc + 1) * SQB], ps[:])
            yield
        if qt % 2 == 1:
            nc.sync.dma_start(
                o3[:, qt - 1 : qt + 1, :], o_sb[:, qt - 1 : qt + 1, :]
            )
            yield

    def oproj_tail(qts):
        for qt in qts:
            for dc in range(2):
                ps = psum_p.tile([P, SQB], F32, tag="pp", name="ps_o")
                for pair in range(2):
                    nc.tensor.matmul(
                        ps[:],
                        lhsT=attnT_sb[:, pair, qt * P : (qt + 1) * P],
                        rhs=woT_sb[:, pair, dc * SQB : (dc + 1) * SQB],
                        start=(pair == 0),
                        stop=(pair == 1),
                    )
                dst = o_sb[:, qt, dc * SQB : (dc + 1) * SQB]
                if dc == 1:
                    nc.scalar.copy(dst, ps[:])  # ACT is idle on the tail
                else:
                    nc.vector.tensor_copy(dst, ps[:])
            nc.sync.dma_start(o3[:, qt : qt + 1, :], o_sb[:, qt : qt + 1, :])

    for _rep in range(repeat):
        load_phase()
        # pair 0: v-proj groups and q/k pair-1 projections ride as fillers
        # inside the attention tile loops. Tag = deadline (must be fully
        # emitted before attn(0, tag) starts).
        for _ in gen_v_group(range(4)):
            pass
        f0 = Filler(
            [
                (1, gen_v_group(range(4, 8))),
                (2, gen_proj_qk(wqT_sb, qT_sb, 1, 0)),
                (2, gen_proj_qk(wkT_sb, kT_sb, 1, 0)),
                (2, gen_v_group(range(8, 12))),
                (3, gen_proj_qk(wqT_sb, qT_sb, 1, 1)),
                (3, gen_proj_qk(wkT_sb, kT_sb, 1, 1)),
                (3, gen_v_group(range(12, 16))),
                (4, gen_proj_qk(wqT_sb, qT_sb, 1, 2)),
                (4, gen_proj_qk(wkT_sb, kT_sb, 1, 2)),
                (4, gen_proj_qk(wqT_sb, qT_sb, 1, 3)),
                (4, gen_proj_qk(wkT_sb, kT_sb, 1, 3)),
            ]
        )
        for sqb in range(N_SQB):
            attention_sqb(0, sqb, filler=f0)
            f0.drain_tag(sqb + 1)
        f0.drain_tag(99)
        # pair 1: oproj rides as filler. Tag = source sqb; a qt range is
        # certain-ready one full sqb after its attnT postproc started.
        f1 = Filler([(qt // 4, gen_oproj_qt(qt)) for qt in range(12)])
        for sqb in range(N_SQB):
            attention_sqb(
                1,
                sqb,
                last=(sqb == N_SQB - 1),
                filler=f1,
                tag_fn=lambda t, s=sqb: (s - 1) if t >= 8 else (s - 2),
            )
            f1.drain_tag(sqb - 1)
        f1.drain_tag(2)
        oproj_tail(range(12, 16))


def make_trimask():
    return np.triu(np.ones((P, P), np.float32)).astype(ml_dtypes.bfloat16)


def prep_core_inputs(x, w_q, w_k, w_v, w_o):
    """Host-side sharding: returns list of 8 in_maps (bf16, pre-transposed)."""
    bf = ml_dtypes.bfloat16
    x = np.asarray(x, np.float32)
    w_q = np.asarray(w_q, np.float32)
    w_k = np.asarray(w_k, np.float32)
    w_v = np.asarray(w_v, np.float32)
    w_o = np.asarray(w_o, np.float32)
    tri = make_trimask()
    ins = []
    for c in range(N_CORES):
        b, g = divmod(c, HG)
        sl = slice(g * DG, (g + 1) * DG)
        ins.append(
            {
                "xT": np.ascontiguousarray(x[b].T).astype(bf),
                "wqT": np.ascontiguousarray(w_q[sl, :].T).astype(bf),
                "wkT": np.ascontiguousarray(w_k[sl, :].T).astype(bf),
                "wvT": np.ascontiguousarray(w_v[sl, :].T).astype(bf),
                "woT": np.ascontiguousarray(w_o[:, sl].T).astype(bf),
                "mask": tri,
            }
        )
    return ins


def combine_outputs(results):
    """results: list of 8 dicts with 'o' [SEQ, D_MODEL] bf16 -> [B, SEQ, D] f32."""
    out = np.zeros((BATCH, SEQ, D_MODEL), np.float32)
    for c, r in enumerate(results):
        out[c // HG] += r["o"].astype(np.float32)
    return out


_NC_CACHE = None


def _get_nc():
    global _NC_CACHE
    if _NC_CACHE is None:
        _NC_CACHE = build_kernel()
    return _NC_CACHE


def kernel(x, w_q, w_k, w_v, w_o):
    """Full-input entry point: shards across 8 NeuronCores, returns full output."""
    from concourse.bass_utils import run_bass_kernel_spmd

    nc = _get_nc()
    in_maps = prep_core_inputs(x, w_q, w_k, w_v, w_o)
    res = run_bass_kernel_spmd(nc, in_maps, core_ids=list(range(N_CORES)))
    return combine_outputs(res.results)
